# revision 4
# baseline (speedup 1.0000x reference)
"""
Trainium2 Bass kernel for nn_BMM_S8T_S8N_S8T:
  y[b,m,n] = sat_i8(round(alpha * sum_k a[b,m,k] * b[b,n,k]))
with a,b int8 [128, 1024, 128], alpha scalar.

Strategy (8 NeuronCores, batch-parallel, 16 batches/core):
 - Host: transpose a,b to [K, M]/[K, N] layout (k on partitions) and convert
   to bf16 (exact for int8). No on-chip transposes or dtype converts at all.
 - Per batch: 16 matmuls (8 m-tiles x 2 n-halves) bf16 x bf16 -> f32 PSUM,
   bit-exact for int8 data.
 - Epilogue per m-tile: one op (ACT activation-copy-with-scale or DVE
   tensor_scalar-mult, greedily balanced by measured op cost 1112:1212)
   does alpha-scale + RNE round + int8 saturate, PSUM -> SBUF. 4-deep PSUM
   rotation keeps both drain engines + PE busy; the drains are the kernel's
   critical resource (f32 PSUM reads are capped at 1 elem/lane/cycle/engine).
 - DMA: thin first loads across sync+scalar queues for fast rampup;
   steady-state loads on sync; stores at half-batch (512KB) granularity
   alternating gpsimd/sync (never the ACT queue mid-kernel - avoids
   head-of-line blocking ACT's epilogue behind store triggers). Last batch
   stores per m-tile-pair to shorten the tail.
"""

import sys

sys.path.insert(0, "/opt/trn_rl_repo")

import numpy as np
import ml_dtypes

N_CORES = 8
B, M, N, K = 128, 1024, 1024, 128
BPC = B // N_CORES  # batches per core
MT = M // 128

ACT_NS = 1112.0  # measured ACTIVATE f32[128,1024] PSUM->SBUF
DVE_NS = 1212.0  # measured TENSOR_SCALAR f32[128,1024] PSUM->SBUF

_cache = {}


def _build(alpha: float):
    import concourse.bacc as bacc
    import concourse.tile as tile
    import concourse.mybir as mybir

    nc = bacc.Bacc("TRN2", target_bir_lowering=False, debug=False)

    bf16 = mybir.dt.bfloat16
    f32 = mybir.dt.float32
    i8 = mybir.dt.int8

    ab_x = nc.dram_tensor("ab_x", [BPC, 2, K, M], bf16, kind="ExternalInput")
    y = nc.dram_tensor("y", [BPC, M, N], i8, kind="ExternalOutput")

    # greedy ACT/DVE assignment balanced by op cost over the whole kernel
    engine_of = []
    act_t = dve_t = 0.0
    for _ in range(BPC * MT):
        if act_t + ACT_NS <= dve_t + DVE_NS:
            engine_of.append("act")
            act_t += ACT_NS
        else:
            engine_of.append("dve")
            dve_t += DVE_NS
    # last drained tile on the faster ACT to shorten the tail
    if engine_of[-1] == "dve":
        for j in range(len(engine_of) - 2, -1, -1):
            if engine_of[j] == "act":
                engine_of[j], engine_of[-1] = "dve", "act"
                break

    with tile.TileContext(nc) as tc:
        with (
            tc.tile_pool(name="inp", bufs=4) as ipool,
            tc.tile_pool(name="outp", bufs=6) as opool,
            tc.tile_pool(name="psmm", bufs=4, space="PSUM") as psmm,
        ):
            abs_ = {}

            def load_batch(bi):
                ab = ipool.tile([128, 2, M], bf16, tag="ab")
                if bi == 0:
                    # thin rampup: 4 x 128KB across two queues; a-halves
                    # first on scalar so MM(0,0) is gated only by a_lo+b_lo
                    nc.scalar.dma_start(out=ab[:, 0, :512], in_=ab_x[0, 0, :, :512])
                    nc.sync.dma_start(out=ab[:, 1, :512], in_=ab_x[0, 1, :, :512])
                    nc.scalar.dma_start(out=ab[:, 0, 512:], in_=ab_x[0, 0, :, 512:])
                    nc.sync.dma_start(out=ab[:, 1, 512:], in_=ab_x[0, 1, :, 512:])
                else:
                    nc.sync.dma_start(
                        out=ab[:], in_=ab_x[bi].rearrange("two k m -> k two m")
                    )
                abs_[bi] = ab

            load_batch(0)
            load_batch(1)

            for bi in range(BPC):
                ab = abs_.pop(bi)
                if bi + 2 < BPC:
                    load_batch(bi + 2)
                last = bi == BPC - 1
                store_eng = nc.gpsimd if bi % 2 == 0 else nc.sync
                nstores = 4 if last else 2
                mt_per_store = MT // nstores
                for sg in range(nstores):
                    y_sb = opool.tile([128, mt_per_store, N], i8, tag=f"y{nstores}")
                    for hm in range(mt_per_store):
                        mt = sg * mt_per_store + hm
                        ps = psmm.tile([128, N], f32, tag="ps")
                        for nh in range(2):
                            nc.tensor.matmul(
                                ps[:, nh * 512 : (nh + 1) * 512],
                                ab[:, 0, mt * 128 : (mt + 1) * 128],
                                ab[:, 1, nh * 512 : (nh + 1) * 512],
                                start=True,
                                stop=True,
                            )
                        if engine_of[bi * MT + mt] == "act":
                            nc.scalar.activation(
                                out=y_sb[:, hm, :],
                                in_=ps[:],
                                func=mybir.ActivationFunctionType.Copy,
                                scale=float(alpha),
                            )
                        else:
                            nc.vector.tensor_scalar(
                                out=y_sb[:, hm, :],
                                in0=ps[:],
                                scalar1=float(alpha),
                                scalar2=None,
                                op0=mybir.AluOpType.mult,
                            )
                    mt0 = sg * mt_per_store
                    dst = y[bi, mt0 * 128 : (mt0 + mt_per_store) * 128, :]
                    se = store_eng if not last else (nc.gpsimd if sg % 2 == 0 else nc.sync)
                    se.dma_start(
                        out=dst.rearrange("(t p) n -> p t n", p=128),
                        in_=y_sb[:],
                    )

    nc.compile()
    return nc


def _get_nc(alpha: float):
    key = float(alpha)
    if key not in _cache:
        _cache[key] = _build(key)
    return _cache[key]


def _prep_inputs(a, b):
    # host-side: [B, M, K] i8 -> [B, K, M] bf16, a||b packed per batch
    bf16 = ml_dtypes.bfloat16
    at = np.ascontiguousarray(a.transpose(0, 2, 1)).astype(bf16)
    bt = np.ascontiguousarray(b.transpose(0, 2, 1)).astype(bf16)
    ab = np.stack([at, bt], axis=1)  # [B, 2, K, M]
    return ab


def kernel(a, b, alpha):
    from concourse.bass_utils import run_bass_kernel_spmd

    a = np.asarray(a)
    b = np.asarray(b)
    assert a.shape == (B, M, K) and a.dtype == np.int8
    assert b.shape == (B, N, K) and b.dtype == np.int8

    nc = _get_nc(float(alpha))

    ab = _prep_inputs(a, b)
    in_maps = [
        {"ab_x": np.ascontiguousarray(ab[c * BPC : (c + 1) * BPC])}
        for c in range(N_CORES)
    ]
    res = run_bass_kernel_spmd(nc, in_maps, list(range(N_CORES)))
    out = np.concatenate([r["y"] for r in res.results], axis=0)
    return out.astype(np.int8)
